# revision 29
# baseline (speedup 1.0000x reference)
"""Multi-head self-attention Trainium2 kernel (B=4, T=2048, D=512, H=8, HD=64).

Sharding: 8 cores = 4 batches x 2 head-groups (4 heads each). Each core:
  - casts x to bf16 (ACT engine), transposes on the PE (bf16, 4 chunks
    chained into one PSUM bank, single wide copy),
  - projects q,k (transposed layout [c, t], bf16) and v (natural [t, e]);
    v is stored in fp8e4m3 with 64 ones-columns per head so the softmax
    denominator comes out of the PV matmul pre-broadcast across 64
    partitions,
  - causal flash attention in "transposed space", software-pipelined so the
    exp (ACT engine) overlaps the S/PV matmuls (PE): S runs two steps ahead
    of PV. exp is computed with bias=-EXPB so P=e^(l-EXPB) fits fp8e4m3
    range (the shift cancels in the softmax ratio). Causal masking of
    diagonal blocks is a multiplicative triangle mask on the DVE. PV runs
    as fp8 DoubleRow matmuls over key-chunk PAIRS (2 contraction tiles per
    instruction at 0.5 cyc/row),
  - normalizes with reciprocal_approx_fast + tensor_mul,
  - output projection (bf16) per query window, deferred into the next
    attention group's pipeline so it never drains the PE.
Host sums the 2 partials per batch (the TP all-reduce of the hint).
"""

import sys

sys.path.insert(0, "/opt/trn_rl_repo")

import numpy as np

import concourse.bass as bass
import concourse.tile as tile
from concourse import bacc, masks, mybir
from concourse.bass_utils import run_bass_kernel_spmd

f32 = mybir.dt.float32
f32r = mybir.dt.float32r
bf16 = mybir.dt.bfloat16
fp8 = mybir.dt.float8e4
u16 = mybir.dt.uint16
u32 = mybir.dt.uint32
DR = mybir.MatmulPerfMode.DoubleRow

B, T, D, H, HD = 4, 2048, 512, 8, 64
NCORES = 8
SCALE = 1.0 / np.sqrt(HD)  # 0.125
EXPB = 4.0  # exp bias shift: P = e^(l - EXPB) fits fp8e4m3 range

_BUILT = None
DEBUG = False


def _build():
    nc = bacc.Bacc("TRN2", target_bir_lowering=False, debug=False)

    x_d = nc.dram_tensor("x", [T, D], f32, kind="ExternalInput")
    wqk_d = nc.dram_tensor("wqk", [D, 512], f32, kind="ExternalInput")
    wv_d = nc.dram_tensor("wv", [D, 256], f32, kind="ExternalInput")
    wo_d = nc.dram_tensor("wo", [256, D], f32, kind="ExternalInput")
    out_d = nc.dram_tensor("out", [T, D], f32, kind="ExternalOutput")
    dbg = {}
    if DEBUG:
        dbg["qkT"] = nc.dram_tensor("dbg_qkT", [128, 4, T], f32, kind="ExternalOutput")
        dbg["vaug"] = nc.dram_tensor(
            "dbg_vaug", [128, 16, 4, 128], f32, kind="ExternalOutput"
        )
        dbg["yTn"] = nc.dram_tensor("dbg_yTn", [128, 2, T], f32, kind="ExternalOutput")
        dbg["P"] = nc.dram_tensor(
            "dbg_P", [128, 2, 2, 512], f32, kind="ExternalOutput"
        )

    with tile.TileContext(nc) as tc:
        with (
            tc.tile_pool(name="big", bufs=1) as big,
            tc.tile_pool(name="xin", bufs=6) as xinp,
            tc.tile_pool(name="pp", bufs=4) as ppool,
            tc.tile_pool(name="rp", bufs=4) as rpool,
            tc.tile_pool(name="op", bufs=3) as opool,
        ):
            # ---- persistent sbuf tensors ----
            xT16 = big.tile([128, 4, 512], bf16)    # [d_part, d_chunk, t<512]
            xT8 = big.tile([128, 4, T], fp8)        # [d_part, d_chunk, t]
            qkT16 = big.tile([128, 4, 512], bf16)   # t4=0: 0,1=q pairs 2,3=k pairs
            qkT8 = big.tile([128, 4, T], fp8)       # all t, fp8
            vaug = big.tile([128, 16, 4, 128], fp8)  # [t_part, t_tile, head, 64 v + 64 ones]
            vaug16 = big.tile([128, 2, 4, 128], bf16)  # bf16 copy of chunks 0,1
            yTn = big.tile([128, 2, T], bf16)       # [c_part, c_chunk, t]
            wqk_f = big.tile([128, 4, 512], f32)
            wv_f = big.tile([128, 4, 256], f32)
            wqk_s = big.tile([128, 4, 512], bf16)
            wv_s = big.tile([128, 4, 256], bf16)
            wqk8 = big.tile([128, 4, 512], fp8)
            wv8 = big.tile([128, 4, 256], fp8)
            wo_f = big.tile([128, 2, 512], f32)
            wo_s = big.tile([128, 2, 512], bf16)

            # ---- identity first (transposes need it), then weight DMAs
            # on the gpsimd ring (engine-issued DMAs occupy the queue for
            # the whole transfer, so they go after the ident build) ----
            identf = big.tile([128, 128], f32)
            masks.make_identity(nc, identf[:])
            ident = big.tile([128, 128], bf16)
            nc.vector.tensor_scalar_mul(ident[:], identf[:], 1.0)
            nc.gpsimd.dma_start(
                wv_f[:], wv_d.ap().rearrange("(c p) m -> p c m", p=128)
            )
            nc.gpsimd.dma_start(
                wqk_f[:], wqk_d.ap().rearrange("(c p) m -> p c m", p=128)
            )
            nc.gpsimd.dma_start(
                wo_f[:], wo_d.ap().rearrange("(c p) m -> p c m", p=128)
            )
            nc.vector.tensor_scalar_mul(wv_s[:], wv_f[:], 1.0)
            nc.vector.tensor_scalar_mul(wv8[:], wv_f[:], 1.0)

            # ---- phase 1+2: load, cast, transpose, project (pipelined) ----
            with tc.tile_pool(name="ps12", bufs=2, space=bass.MemorySpace.PSUM) as ps12:
                for tt in range(16):
                    t0, t1 = tt * 128, (tt + 1) * 128
                    xt = xinp.tile([128, 512], f32, tag="xt", name=f"xt{tt}")
                    nc.sync.dma_start(xt[:], x_d.ap()[t0:t1, :])
                    x16 = xinp.tile([128, 512], bf16, tag="x16", bufs=3)
                    nc.scalar.copy(x16[:], xt[:])
                    px = ps12.tile([128, 512], bf16, tag="x", bufs=3)
                    for c in range(4):
                        # 4 transposes share one PSUM bank; start=True would
                        # zero the whole 2KB bank, so chain as one group.
                        nc.tensor.matmul(
                            px[:, c * 128:(c + 1) * 128],
                            x16[:, c * 128:(c + 1) * 128],
                            ident[:],
                            is_transpose=True,
                            start=(c == 0),
                            stop=(c == 3),
                            skip_group_check=True,
                        )
                    if tt < 4:
                        nc.vector.tensor_scalar_mul(
                            xT16[:, :, t0:t1],
                            px[:].rearrange("p (c q) -> p c q", c=4),
                            1.0,
                        )
                    nc.vector.tensor_scalar_mul(
                        xT8[:, :, t0:t1],
                        px[:].rearrange("p (c q) -> p c q", c=4),
                        1.0,
                    )
                    pv = ps12.tile([128, 256], f32, tag="v", bufs=2)
                    if tt < 4:
                        for c in range(4):
                            nc.tensor.matmul(
                                pv[:],
                                xT16[:, c, t0:t1],
                                wv_s[:, c, :],
                                start=(c == 0),
                                stop=(c == 3),
                            )
                    else:
                        for c2 in range(2):
                            nc.tensor.matmul(
                                pv[:],
                                xT8[:, 2 * c2:2 * c2 + 2, t0:t1],
                                wv8[:, 2 * c2:2 * c2 + 2, :],
                                start=(c2 == 0),
                                stop=(c2 == 1),
                                perf_mode=DR,
                            )
                    nc.vector.tensor_scalar_mul(
                        vaug[:, tt, :, 0:64],
                        pv[:].rearrange("p (h e) -> p h e", e=64),
                        1.0,
                    )
                    # fp8e4m3 1.0 = 0x38
                    nc.gpsimd.memset(
                        vaug[:, tt, :, 64:128].bitcast(u32), 0x38383838
                    )
                    if tt < 2:
                        nc.vector.tensor_scalar_mul(
                            vaug16[:, tt, :, 0:64],
                            pv[:].rearrange("p (h e) -> p h e", e=64),
                            1.0,
                        )
                        nc.gpsimd.memset(
                            vaug16[:, tt, :, 64:128].bitcast(u32), 0x3F803F80
                        )
                    if tt == 1:
                        nc.vector.tensor_scalar_mul(wqk_s[:], wqk_f[:], 1.0)
                        nc.vector.tensor_scalar_mul(wqk8[:], wqk_f[:], 1.0)
                    if tt % 4 == 3:
                        t4 = tt // 4
                        for ct in range(4):
                            pqk = ps12.tile([128, 512], f32, tag="qk", bufs=2)
                            if t4 == 0:
                                for c in range(4):
                                    nc.tensor.matmul(
                                        pqk[:],
                                        wqk_s[:, c, ct * 128:(ct + 1) * 128],
                                        xT16[:, c, 0:512],
                                        start=(c == 0),
                                        stop=(c == 3),
                                    )
                                nc.scalar.copy(qkT16[:, ct, :], pqk[:])
                                nc.vector.tensor_scalar_mul(
                                    qkT8[:, ct, 0:512], pqk[:], 1.0
                                )
                            else:
                                for c2 in range(2):
                                    nc.tensor.matmul(
                                        pqk[:],
                                        wqk8[:, 2 * c2:2 * c2 + 2,
                                             ct * 128:(ct + 1) * 128],
                                        xT8[:, 2 * c2:2 * c2 + 2,
                                            t4 * 512:(t4 + 1) * 512],
                                        start=(c2 == 0),
                                        stop=(c2 == 1),
                                        perf_mode=DR,
                                    )
                                nc.scalar.copy(
                                    qkT8[:, ct, t4 * 512:(t4 + 1) * 512], pqk[:]
                                )

            # ---- constants for phase 3 (deferred off the startup path) ----
            tri2f = big.tile([128, 2, 128], f32)  # [k, copy, q] = 1.0 where q >= k
            masks.make_upper_triangular(nc, tri2f[:, 0, :], val=1.0, diag=True)
            masks.make_upper_triangular(nc, tri2f[:, 1, :], val=1.0, diag=True)
            tri2 = big.tile([128, 2, 128], fp8)
            nc.vector.tensor_scalar_mul(tri2[:], tri2f[:], 1.0)
            tri2b = big.tile([128, 2, 128], bf16)
            nc.vector.tensor_scalar_mul(tri2b[:], tri2f[:], 1.0)
            expb = big.tile([128, 1], f32)
            nc.gpsimd.memset(expb[:], -EXPB)
            nc.vector.tensor_scalar_mul(wo_s[:], wo_f[:], 1.0)

            # ---- phase 3+4: attention + output projection, flat stream ----
            with (
                tc.tile_pool(name="psS", bufs=2, space=bass.MemorySpace.PSUM) as psS,
                tc.tile_pool(name="psPV", bufs=2, space=bass.MemorySpace.PSUM) as psPV,
            ):
                def emit_oproj_tt(tt, ring):
                    po = psPV.tile(
                        [128, 512], f32, tag=("pvA" if ring % 2 == 0 else "pvB"),
                        name="po",
                    )
                    nc.tensor.matmul(
                        po[:], yTn[:, 0, tt * 128:(tt + 1) * 128],
                        wo_s[:, 0, :], start=True, stop=False,
                    )
                    nc.tensor.matmul(
                        po[:], yTn[:, 1, tt * 128:(tt + 1) * 128],
                        wo_s[:, 1, :], start=False, stop=True,
                    )
                    ot = opool.tile([128, 512], f32, tag="o")
                    nc.vector.tensor_scalar_mul(ot[:], po[:], 1.0)
                    nc.sync.dma_start(
                        out_d.ap()[tt * 128:(tt + 1) * 128, :], ot[:]
                    )

                def width(it, n):
                    s = n - 4 * it
                    if s < 0:
                        return 512, 512
                    W = 512 - 128 * s
                    return W, max(W, 256)

                gstate = {}

                def do_S(it, p, n):
                    st = gstate[(it, p)]
                    if n == 0:
                        st["pvA"] = psPV.tile(
                            [128, 512], f32, tag="pvA", name="pvA"
                        )
                        st["pvB"] = psPV.tile(
                            [128, 512], f32, tag="pvB", name="pvB"
                        )
                    i0 = it * 512
                    W, Wc = width(it, n)
                    sab = psS.tile([128, 1024], f32, tag="S", name="sab")
                    src_qk = qkT16 if it == 0 else qkT8
                    kA = src_qk[0:64, 2 + p, n * 128:(n + 1) * 128]
                    kB = src_qk[64:128, 2 + p, n * 128:(n + 1) * 128]
                    qA = src_qk[0:64, p, i0 + 512 - Wc:i0 + 512]
                    qB = src_qk[64:128, p, i0 + 512 - Wc:i0 + 512]
                    nc.tensor.matmul(
                        sab[:, 512 - Wc:512], kA, qA,
                        start=True, stop=True, tile_position=(0, 0),
                    )
                    nc.tensor.matmul(
                        sab[:, 1024 - Wc:1024], kB, qB,
                        start=True, stop=True, tile_position=(64, 0),
                    )
                    st["sabs"][n] = sab

                def do_exp(it, p, n):
                    st = gstate[(it, p)]
                    W, Wc = width(it, n)
                    sab = st["sabs"].pop(n)
                    hi = it == 0 and n < 2  # bf16 early-query path
                    if n % 2 == 0:
                        if hi:
                            st["pab2s"][n // 2] = ppool.tile(
                                [128, 2, 2, 512], bf16, tag="P16",
                                name="pab16", bufs=2,
                            )
                        else:
                            st["pab2s"][n // 2] = ppool.tile(
                                [128, 2, 2, 512], fp8, tag="P", name="pab2"
                            )
                    pab2 = st["pab2s"][n // 2]
                    j = n % 2
                    sl_in = sab[:].rearrange("p (two w) -> p two w", two=2)[
                        :, :, 512 - Wc:512
                    ]
                    sl_out = pab2[:, j, :, 512 - Wc:512]
                    nc.scalar.activation(
                        sl_out, sl_in,
                        mybir.ActivationFunctionType.Exp,
                        bias=expb[:], scale=SCALE,
                    )
                    s = n - 4 * it
                    if s >= 0:
                        W0 = 512 - W
                        m = pab2[:, j, :, W0:W0 + 128]
                        nc.vector.tensor_mul(m, m, tri2b[:] if hi else tri2[:])
                        if Wc > W:  # s == 3: zero the pad block
                            nc.gpsimd.memset(
                                pab2[:, j, :, 256:384].bitcast(u16), 0
                            )
                        if s == 1:  # pair partner runs full-width
                            nc.gpsimd.memset(
                                pab2[:, j, :, 0:128].bitcast(
                                    u32 if hi else u16
                                ), 0
                            )
                    if DEBUG and p == 0 and it == 0 and n == 1:
                        nc.gpsimd.dma_start(dbg["P"].ap(), pab2[:])

                def do_PV(it, p, m):
                    st = gstate[(it, p)]
                    njc = 4 * it + 4
                    n0 = 2 * m
                    _, Wc0 = width(it, n0)
                    _, Wc1 = width(it, n0 + 1)
                    Wc = max(Wc0, Wc1)
                    pab2 = st["pab2s"].pop(m)
                    pvA, pvB = st["pvA"], st["pvB"]
                    if it == 0 and m == 0:
                        # bf16 early-query path: plain matmuls
                        for j in range(2):
                            nc.tensor.matmul(
                                pvA[:],
                                vaug16[:, j, 2 * p, :],
                                pab2[:, j, 0, :],
                                start=(j == 0), stop=False,
                                skip_group_check=True,
                            )
                            nc.tensor.matmul(
                                pvB[:],
                                vaug16[:, j, 2 * p + 1, :],
                                pab2[:, j, 1, :],
                                start=(j == 0), stop=False,
                                skip_group_check=True,
                            )
                        return
                    # fp8 DoubleRow over the key-chunk pair (2m, 2m+1)
                    nc.tensor.matmul(
                        pvA[:, 512 - Wc:512],
                        vaug[:, n0:n0 + 2, 2 * p, :],
                        pab2[:, :, 0, 512 - Wc:512],
                        start=(m == 0), stop=(m == njc // 2 - 1),
                        perf_mode=DR, skip_group_check=True,
                    )
                    nc.tensor.matmul(
                        pvB[:, 512 - Wc:512],
                        vaug[:, n0:n0 + 2, 2 * p + 1, :],
                        pab2[:, :, 1, 512 - Wc:512],
                        start=(m == 0), stop=(m == njc // 2 - 1),
                        perf_mode=DR, skip_group_check=True,
                    )

                def do_norm(it, p):
                    st = gstate[(it, p)]
                    i0 = it * 512
                    pvA, pvB = st["pvA"], st["pvB"]
                    # normalize: rows 64:128 hold l replicated 64x.
                    # reciprocal_approx_fast needs SBUF input.
                    lA = rpool.tile([64, 512], f32, tag="l")
                    nc.vector.tensor_scalar_mul(lA[:], pvA[64:128, :], 1.0)
                    rA = rpool.tile([64, 512], f32, tag="r")
                    nc.vector.reciprocal_approx_fast(rA[:], lA[:])
                    nc.vector.tensor_mul(
                        yTn[0:64, p, i0:i0 + 512], pvA[0:64, :], rA[:]
                    )
                    lB = rpool.tile([64, 512], f32, tag="l")
                    nc.vector.tensor_scalar_mul(lB[:], pvB[64:128, :], 1.0)
                    rB = rpool.tile([64, 512], f32, tag="r")
                    nc.vector.reciprocal_approx_fast(rB[:], lB[:])
                    nc.vector.tensor_mul(
                        yTn[64:128, p, i0:i0 + 512], pvB[0:64, :], rB[:]
                    )

                its_order = [1, 2, 3, 0]
                items = []
                for it in its_order:
                    for p in range(2):
                        gstate[(it, p)] = {"sabs": {}, "pab2s": {}}
                        for n in range(4 * it + 4):
                            items.append((it, p, n))

                NI = len(items)
                oproj_queue = []  # (emit_at_step, tt)
                for i in range(NI + 2):
                    while oproj_queue and oproj_queue[0][0] <= i:
                        _, tt, ring = oproj_queue.pop(0)
                        emit_oproj_tt(tt, ring)
                    if i < NI:
                        do_S(*items[i])
                    if 1 <= i <= NI:
                        do_exp(*items[i - 1])
                    if i >= 2:
                        it, p, n = items[i - 2]
                        if n % 2 == 1:
                            do_PV(it, p, n // 2)
                        if n == 4 * it + 3:
                            do_norm(it, p)
                            if p == 1:
                                step = 2 if it == its_order[-1] else 4
                                spread = 1 if it == its_order[-1] else 2
                                for k, tt in enumerate(
                                    range(4 * it, 4 * it + 4)
                                ):
                                    oproj_queue.append(
                                        (i + step + spread * k, tt, k)
                                    )
                for _, tt, ring in oproj_queue:
                    emit_oproj_tt(tt, ring)

                if DEBUG:
                    nc.gpsimd.dma_start(dbg["qkT"].ap(), qkT8[:])
                    nc.gpsimd.dma_start(dbg["vaug"].ap(), vaug[:])
                    nc.gpsimd.dma_start(dbg["yTn"].ap(), yTn[:])

    nc.compile()
    return nc


def _get_nc():
    global _BUILT
    if _BUILT is None:
        _BUILT = _build()
    return _BUILT


def _make_in_maps(x, Wqkv, Wout):
    q, k, v = Wqkv[:, 0:512], Wqkv[:, 512:1024], Wqkv[:, 1024:1536]
    in_maps = []
    for core in range(NCORES):
        b, g = core // 2, core % 2
        hs = [g * 4 + i for i in range(4)]
        wqk = np.concatenate(
            [q[:, h * 64:(h + 1) * 64] for h in hs]
            + [k[:, h * 64:(h + 1) * 64] for h in hs],
            axis=1,
        )
        wv = np.ascontiguousarray(v[:, g * 256:(g + 1) * 256])
        wo = np.ascontiguousarray(Wout[g * 256:(g + 1) * 256, :])
        in_maps.append(
            {
                "x": np.ascontiguousarray(x[b]),
                "wqk": np.ascontiguousarray(wqk),
                "wv": wv,
                "wo": wo,
            }
        )
    return in_maps


def _run(x, Wqkv, Wout, trace=False):
    nc = _get_nc()
    in_maps = _make_in_maps(x, Wqkv, Wout)
    res = run_bass_kernel_spmd(
        nc, in_maps, core_ids=list(range(NCORES)), trace=trace
    )
    out = np.empty((B, T, D), dtype=np.float32)
    for b in range(B):
        out[b] = res.results[2 * b]["out"] + res.results[2 * b + 1]["out"]
    return out, res


def _reference_fallback(x, attn_mask, Wqkv, Wout):
    # general (non-causal-mask) path: plain numpy
    qkv = x @ Wqkv
    q, k, v = np.split(qkv, 3, axis=-1)

    def heads(t):
        return t.reshape(B, T, H, HD).transpose(0, 2, 1, 3)

    q, k, v = heads(q), heads(k), heads(v)
    att = np.einsum("bhqd,bhkd->bhqk", q, k) * SCALE
    att = np.where(attn_mask[None, None] == 0, -np.inf, att)
    att = att - att.max(axis=-1, keepdims=True)
    att = np.exp(att)
    att = att / att.sum(axis=-1, keepdims=True)
    y = np.einsum("bhqk,bhkd->bhqd", att, v)
    return (y.transpose(0, 2, 1, 3).reshape(B, T, D) @ Wout).astype(np.float32)


def kernel(x, attn_mask, Wqkv, Wout):
    x = np.asarray(x, dtype=np.float32)
    attn_mask = np.asarray(attn_mask)
    Wqkv = np.asarray(Wqkv, dtype=np.float32)
    Wout = np.asarray(Wout, dtype=np.float32)

    causal = bool(
        np.array_equal(attn_mask != 0, np.tril(np.ones((T, T), dtype=bool)))
    )
    if not causal:
        return _reference_fallback(x, attn_mask, Wqkv, Wout)

    out, _ = _run(x, Wqkv, Wout, trace=False)
    return out


# revision 30
# speedup vs baseline: 1.0192x; 1.0192x over previous
"""Multi-head self-attention Trainium2 kernel (B=4, T=2048, D=512, H=8, HD=64).

Sharding: 8 cores = 4 batches x 2 head-groups (4 heads each). Each core:
  - casts x to bf16 (ACT engine), transposes on the PE (bf16, 4 chunks
    chained into one PSUM bank, single wide copy),
  - projects q,k (transposed layout [c, t], bf16) and v (natural [t, e]);
    v is stored in fp8e4m3 with 64 ones-columns per head so the softmax
    denominator comes out of the PV matmul pre-broadcast across 64
    partitions,
  - causal flash attention in "transposed space", software-pipelined so the
    exp (ACT engine) overlaps the S/PV matmuls (PE): S runs two steps ahead
    of PV. exp is computed with bias=-EXPB so P=e^(l-EXPB) fits fp8e4m3
    range (the shift cancels in the softmax ratio). Causal masking of
    diagonal blocks is a multiplicative triangle mask on the DVE. PV runs
    as fp8 DoubleRow matmuls over key-chunk PAIRS (2 contraction tiles per
    instruction at 0.5 cyc/row),
  - normalizes with reciprocal_approx_fast + tensor_mul,
  - output projection (bf16) per query window, deferred into the next
    attention group's pipeline so it never drains the PE.
Host sums the 2 partials per batch (the TP all-reduce of the hint).
"""

import sys

sys.path.insert(0, "/opt/trn_rl_repo")

import numpy as np

import concourse.bass as bass
import concourse.tile as tile
from concourse import bacc, masks, mybir
from concourse.bass_utils import run_bass_kernel_spmd

f32 = mybir.dt.float32
f32r = mybir.dt.float32r
bf16 = mybir.dt.bfloat16
fp8 = mybir.dt.float8e4
u16 = mybir.dt.uint16
u32 = mybir.dt.uint32
DR = mybir.MatmulPerfMode.DoubleRow

B, T, D, H, HD = 4, 2048, 512, 8, 64
NCORES = 8
SCALE = 1.0 / np.sqrt(HD)  # 0.125
EXPB = 4.0  # exp bias shift: P = e^(l - EXPB) fits fp8e4m3 range

_BUILT = None
DEBUG = False


def _build():
    nc = bacc.Bacc("TRN2", target_bir_lowering=False, debug=False)

    x_d = nc.dram_tensor("x", [T, D], f32, kind="ExternalInput")
    wqk_d = nc.dram_tensor("wqk", [D, 512], f32, kind="ExternalInput")
    wv_d = nc.dram_tensor("wv", [D, 256], f32, kind="ExternalInput")
    wo_d = nc.dram_tensor("wo", [256, D], f32, kind="ExternalInput")
    out_d = nc.dram_tensor("out", [T, D], f32, kind="ExternalOutput")
    dbg = {}
    if DEBUG:
        dbg["qkT"] = nc.dram_tensor("dbg_qkT", [128, 4, T], f32, kind="ExternalOutput")
        dbg["vaug"] = nc.dram_tensor(
            "dbg_vaug", [128, 16, 4, 128], f32, kind="ExternalOutput"
        )
        dbg["yTn"] = nc.dram_tensor("dbg_yTn", [128, 2, T], f32, kind="ExternalOutput")
        dbg["P"] = nc.dram_tensor(
            "dbg_P", [128, 2, 2, 512], f32, kind="ExternalOutput"
        )

    with tile.TileContext(nc) as tc:
        with (
            tc.tile_pool(name="big", bufs=1) as big,
            tc.tile_pool(name="xin", bufs=6) as xinp,
            tc.tile_pool(name="pp", bufs=4) as ppool,
            tc.tile_pool(name="rp", bufs=4) as rpool,
            tc.tile_pool(name="op", bufs=3) as opool,
        ):
            # ---- persistent sbuf tensors ----
            xT = big.tile([128, 4, T], bf16)        # [d_part, d_chunk, t]
            qkT = big.tile([128, 4, T], bf16)       # ct: 0=q(h0,h1) 1=q(h2,h3) 2=k(h0,h1) 3=k(h2,h3)
            vaug = big.tile([128, 16, 4, 128], fp8)  # [t_part, t_tile, head, 64 v + 64 ones]
            vaug16 = big.tile([128, 2, 4, 128], bf16)  # bf16 copy of chunks 0,1
            yTn = big.tile([128, 2, T], bf16)       # [c_part, c_chunk, t]
            wqk_f = big.tile([128, 4, 512], f32)
            wv_f = big.tile([128, 4, 256], f32)
            wqk_s = big.tile([128, 4, 512], bf16)
            wv_s = big.tile([128, 4, 256], bf16)
            wo_f = big.tile([128, 2, 512], f32)
            wo_s = big.tile([128, 2, 512], bf16)

            # ---- identity first (transposes need it), then weight DMAs
            # on the gpsimd ring (engine-issued DMAs occupy the queue for
            # the whole transfer, so they go after the ident build) ----
            identf = big.tile([128, 128], f32)
            masks.make_identity(nc, identf[:])
            ident = big.tile([128, 128], bf16)
            nc.vector.tensor_scalar_mul(ident[:], identf[:], 1.0)
            nc.gpsimd.dma_start(
                wv_f[:], wv_d.ap().rearrange("(c p) m -> p c m", p=128)
            )
            nc.gpsimd.dma_start(
                wqk_f[:], wqk_d.ap().rearrange("(c p) m -> p c m", p=128)
            )
            nc.gpsimd.dma_start(
                wo_f[:], wo_d.ap().rearrange("(c p) m -> p c m", p=128)
            )
            nc.vector.tensor_scalar_mul(wv_s[:], wv_f[:], 1.0)

            # ---- phase 1+2: load, cast, transpose, project (pipelined) ----
            with tc.tile_pool(name="ps12", bufs=2, space=bass.MemorySpace.PSUM) as ps12:
                for tt in range(16):
                    t0, t1 = tt * 128, (tt + 1) * 128
                    xt = xinp.tile([128, 512], f32, tag="xt", name=f"xt{tt}")
                    nc.sync.dma_start(xt[:], x_d.ap()[t0:t1, :])
                    x16 = xinp.tile([128, 512], bf16, tag="x16", bufs=3)
                    nc.scalar.copy(x16[:], xt[:])
                    px = ps12.tile([128, 512], bf16, tag="x", bufs=3)
                    for c in range(4):
                        # 4 transposes share one PSUM bank; start=True would
                        # zero the whole 2KB bank, so chain as one group.
                        nc.tensor.matmul(
                            px[:, c * 128:(c + 1) * 128],
                            x16[:, c * 128:(c + 1) * 128],
                            ident[:],
                            is_transpose=True,
                            start=(c == 0),
                            stop=(c == 3),
                            skip_group_check=True,
                        )
                    nc.vector.tensor_scalar_mul(
                        xT[:, :, t0:t1],
                        px[:].rearrange("p (c q) -> p c q", c=4),
                        1.0,
                    )
                    pv = ps12.tile([128, 256], f32, tag="v", bufs=2)
                    for c in range(4):
                        nc.tensor.matmul(
                            pv[:],
                            xT[:, c, t0:t1],
                            wv_s[:, c, :],
                            start=(c == 0),
                            stop=(c == 3),
                        )
                    nc.vector.tensor_scalar_mul(
                        vaug[:, tt, :, 0:64],
                        pv[:].rearrange("p (h e) -> p h e", e=64),
                        1.0,
                    )
                    # fp8e4m3 1.0 = 0x38
                    nc.gpsimd.memset(
                        vaug[:, tt, :, 64:128].bitcast(u32), 0x38383838
                    )
                    if tt < 2:
                        nc.vector.tensor_scalar_mul(
                            vaug16[:, tt, :, 0:64],
                            pv[:].rearrange("p (h e) -> p h e", e=64),
                            1.0,
                        )
                        nc.gpsimd.memset(
                            vaug16[:, tt, :, 64:128].bitcast(u32), 0x3F803F80
                        )
                    if tt == 1:
                        nc.vector.tensor_scalar_mul(wqk_s[:], wqk_f[:], 1.0)
                    if tt % 4 == 3:
                        t4 = tt // 4
                        for ct in range(4):
                            pqk = ps12.tile([128, 512], f32, tag="qk", bufs=2)
                            for c in range(4):
                                nc.tensor.matmul(
                                    pqk[:],
                                    wqk_s[:, c, ct * 128:(ct + 1) * 128],
                                    xT[:, c, t4 * 512:(t4 + 1) * 512],
                                    start=(c == 0),
                                    stop=(c == 3),
                                )
                            nc.vector.tensor_scalar_mul(
                                qkT[:, ct, t4 * 512:(t4 + 1) * 512], pqk[:], 1.0
                            )

            # ---- constants for phase 3 (deferred off the startup path) ----
            tri2f = big.tile([128, 2, 128], f32)  # [k, copy, q] = 1.0 where q >= k
            masks.make_upper_triangular(nc, tri2f[:, 0, :], val=1.0, diag=True)
            masks.make_upper_triangular(nc, tri2f[:, 1, :], val=1.0, diag=True)
            tri2 = big.tile([128, 2, 128], fp8)
            nc.vector.tensor_scalar_mul(tri2[:], tri2f[:], 1.0)
            tri2b = big.tile([128, 2, 128], bf16)
            nc.vector.tensor_scalar_mul(tri2b[:], tri2f[:], 1.0)
            expb = big.tile([128, 1], f32)
            nc.gpsimd.memset(expb[:], -EXPB)
            nc.vector.tensor_scalar_mul(wo_s[:], wo_f[:], 1.0)

            # ---- phase 3+4: attention + output projection, flat stream ----
            with (
                tc.tile_pool(name="psS", bufs=2, space=bass.MemorySpace.PSUM) as psS,
                tc.tile_pool(name="psPV", bufs=2, space=bass.MemorySpace.PSUM) as psPV,
            ):
                def emit_oproj_tt(tt, ring):
                    po = psPV.tile(
                        [128, 512], f32, tag=("pvA" if ring % 2 == 0 else "pvB"),
                        name="po",
                    )
                    nc.tensor.matmul(
                        po[:], yTn[:, 0, tt * 128:(tt + 1) * 128],
                        wo_s[:, 0, :], start=True, stop=False,
                    )
                    nc.tensor.matmul(
                        po[:], yTn[:, 1, tt * 128:(tt + 1) * 128],
                        wo_s[:, 1, :], start=False, stop=True,
                    )
                    ot = opool.tile([128, 512], f32, tag="o")
                    nc.vector.tensor_scalar_mul(ot[:], po[:], 1.0)
                    nc.sync.dma_start(
                        out_d.ap()[tt * 128:(tt + 1) * 128, :], ot[:]
                    )

                def width(it, n):
                    s = n - 4 * it
                    if s < 0:
                        return 512, 512
                    W = 512 - 128 * s
                    return W, max(W, 256)

                gstate = {}

                def do_S(it, p, n):
                    st = gstate[(it, p)]
                    if n == 0:
                        st["pvA"] = psPV.tile(
                            [128, 512], f32, tag="pvA", name="pvA"
                        )
                        st["pvB"] = psPV.tile(
                            [128, 512], f32, tag="pvB", name="pvB"
                        )
                    i0 = it * 512
                    W, Wc = width(it, n)
                    sab = psS.tile([128, 1024], f32, tag="S", name="sab")
                    kA = qkT[0:64, 2 + p, n * 128:(n + 1) * 128]
                    kB = qkT[64:128, 2 + p, n * 128:(n + 1) * 128]
                    qA = qkT[0:64, p, i0 + 512 - Wc:i0 + 512]
                    qB = qkT[64:128, p, i0 + 512 - Wc:i0 + 512]
                    nc.tensor.matmul(
                        sab[:, 512 - Wc:512], kA, qA,
                        start=True, stop=True, tile_position=(0, 0),
                    )
                    nc.tensor.matmul(
                        sab[:, 1024 - Wc:1024], kB, qB,
                        start=True, stop=True, tile_position=(64, 0),
                    )
                    st["sabs"][n] = sab

                def do_exp(it, p, n):
                    st = gstate[(it, p)]
                    W, Wc = width(it, n)
                    sab = st["sabs"].pop(n)
                    hi = it == 0 and n < 2  # bf16 early-query path
                    if n % 2 == 0:
                        if hi:
                            st["pab2s"][n // 2] = ppool.tile(
                                [128, 2, 2, 512], bf16, tag="P16",
                                name="pab16", bufs=2,
                            )
                        else:
                            st["pab2s"][n // 2] = ppool.tile(
                                [128, 2, 2, 512], fp8, tag="P", name="pab2"
                            )
                    pab2 = st["pab2s"][n // 2]
                    j = n % 2
                    sl_in = sab[:].rearrange("p (two w) -> p two w", two=2)[
                        :, :, 512 - Wc:512
                    ]
                    sl_out = pab2[:, j, :, 512 - Wc:512]
                    nc.scalar.activation(
                        sl_out, sl_in,
                        mybir.ActivationFunctionType.Exp,
                        bias=expb[:], scale=SCALE,
                    )
                    s = n - 4 * it
                    if s >= 0:
                        W0 = 512 - W
                        m = pab2[:, j, :, W0:W0 + 128]
                        nc.vector.tensor_mul(m, m, tri2b[:] if hi else tri2[:])
                        if Wc > W:  # s == 3: zero the pad block
                            nc.gpsimd.memset(
                                pab2[:, j, :, 256:384].bitcast(u16), 0
                            )
                        if s == 1:  # pair partner runs full-width
                            nc.gpsimd.memset(
                                pab2[:, j, :, 0:128].bitcast(
                                    u32 if hi else u16
                                ), 0
                            )
                    if DEBUG and p == 0 and it == 0 and n == 1:
                        nc.gpsimd.dma_start(dbg["P"].ap(), pab2[:])

                def do_PV(it, p, m):
                    st = gstate[(it, p)]
                    njc = 4 * it + 4
                    n0 = 2 * m
                    _, Wc0 = width(it, n0)
                    _, Wc1 = width(it, n0 + 1)
                    Wc = max(Wc0, Wc1)
                    pab2 = st["pab2s"].pop(m)
                    pvA, pvB = st["pvA"], st["pvB"]
                    if it == 0 and m == 0:
                        # bf16 early-query path: plain matmuls
                        for j in range(2):
                            nc.tensor.matmul(
                                pvA[:],
                                vaug16[:, j, 2 * p, :],
                                pab2[:, j, 0, :],
                                start=(j == 0), stop=False,
                                skip_group_check=True,
                            )
                            nc.tensor.matmul(
                                pvB[:],
                                vaug16[:, j, 2 * p + 1, :],
                                pab2[:, j, 1, :],
                                start=(j == 0), stop=False,
                                skip_group_check=True,
                            )
                        return
                    # fp8 DoubleRow over the key-chunk pair (2m, 2m+1)
                    nc.tensor.matmul(
                        pvA[:, 512 - Wc:512],
                        vaug[:, n0:n0 + 2, 2 * p, :],
                        pab2[:, :, 0, 512 - Wc:512],
                        start=(m == 0), stop=(m == njc // 2 - 1),
                        perf_mode=DR, skip_group_check=True,
                    )
                    nc.tensor.matmul(
                        pvB[:, 512 - Wc:512],
                        vaug[:, n0:n0 + 2, 2 * p + 1, :],
                        pab2[:, :, 1, 512 - Wc:512],
                        start=(m == 0), stop=(m == njc // 2 - 1),
                        perf_mode=DR, skip_group_check=True,
                    )

                def do_norm(it, p):
                    st = gstate[(it, p)]
                    i0 = it * 512
                    pvA, pvB = st["pvA"], st["pvB"]
                    # normalize: rows 64:128 hold l replicated 64x.
                    # reciprocal_approx_fast needs SBUF input.
                    lA = rpool.tile([64, 512], f32, tag="l")
                    nc.vector.tensor_scalar_mul(lA[:], pvA[64:128, :], 1.0)
                    rA = rpool.tile([64, 512], f32, tag="r")
                    nc.vector.reciprocal_approx_fast(rA[:], lA[:])
                    nc.vector.tensor_mul(
                        yTn[0:64, p, i0:i0 + 512], pvA[0:64, :], rA[:]
                    )
                    lB = rpool.tile([64, 512], f32, tag="l")
                    nc.vector.tensor_scalar_mul(lB[:], pvB[64:128, :], 1.0)
                    rB = rpool.tile([64, 512], f32, tag="r")
                    nc.vector.reciprocal_approx_fast(rB[:], lB[:])
                    nc.vector.tensor_mul(
                        yTn[64:128, p, i0:i0 + 512], pvB[0:64, :], rB[:]
                    )

                its_order = [1, 2, 3, 0]
                items = []
                for it in its_order:
                    for p in range(2):
                        gstate[(it, p)] = {"sabs": {}, "pab2s": {}}
                        for n in range(4 * it + 4):
                            items.append((it, p, n))

                NI = len(items)
                oproj_queue = []  # (emit_at_step, tt)
                for i in range(NI + 2):
                    while oproj_queue and oproj_queue[0][0] <= i:
                        _, tt, ring = oproj_queue.pop(0)
                        emit_oproj_tt(tt, ring)
                    if i < NI:
                        do_S(*items[i])
                    if 1 <= i <= NI:
                        do_exp(*items[i - 1])
                    if i >= 2:
                        it, p, n = items[i - 2]
                        if n % 2 == 1:
                            do_PV(it, p, n // 2)
                        if n == 4 * it + 3:
                            do_norm(it, p)
                            if p == 1:
                                step = 2 if it == its_order[-1] else 4
                                spread = 1 if it == its_order[-1] else 2
                                for k, tt in enumerate(
                                    range(4 * it, 4 * it + 4)
                                ):
                                    oproj_queue.append(
                                        (i + step + spread * k, tt, k)
                                    )
                for _, tt, ring in oproj_queue:
                    emit_oproj_tt(tt, ring)

                if DEBUG:
                    nc.gpsimd.dma_start(dbg["qkT"].ap(), qkT[:])
                    nc.gpsimd.dma_start(dbg["vaug"].ap(), vaug[:])
                    nc.gpsimd.dma_start(dbg["yTn"].ap(), yTn[:])

    nc.compile()
    return nc


def _get_nc():
    global _BUILT
    if _BUILT is None:
        _BUILT = _build()
    return _BUILT


def _make_in_maps(x, Wqkv, Wout):
    q, k, v = Wqkv[:, 0:512], Wqkv[:, 512:1024], Wqkv[:, 1024:1536]
    in_maps = []
    for core in range(NCORES):
        b, g = core // 2, core % 2
        hs = [g * 4 + i for i in range(4)]
        wqk = np.concatenate(
            [q[:, h * 64:(h + 1) * 64] for h in hs]
            + [k[:, h * 64:(h + 1) * 64] for h in hs],
            axis=1,
        )
        wv = np.ascontiguousarray(v[:, g * 256:(g + 1) * 256])
        wo = np.ascontiguousarray(Wout[g * 256:(g + 1) * 256, :])
        in_maps.append(
            {
                "x": np.ascontiguousarray(x[b]),
                "wqk": np.ascontiguousarray(wqk),
                "wv": wv,
                "wo": wo,
            }
        )
    return in_maps


def _run(x, Wqkv, Wout, trace=False):
    nc = _get_nc()
    in_maps = _make_in_maps(x, Wqkv, Wout)
    res = run_bass_kernel_spmd(
        nc, in_maps, core_ids=list(range(NCORES)), trace=trace
    )
    out = np.empty((B, T, D), dtype=np.float32)
    for b in range(B):
        out[b] = res.results[2 * b]["out"] + res.results[2 * b + 1]["out"]
    return out, res


def _reference_fallback(x, attn_mask, Wqkv, Wout):
    # general (non-causal-mask) path: plain numpy
    qkv = x @ Wqkv
    q, k, v = np.split(qkv, 3, axis=-1)

    def heads(t):
        return t.reshape(B, T, H, HD).transpose(0, 2, 1, 3)

    q, k, v = heads(q), heads(k), heads(v)
    att = np.einsum("bhqd,bhkd->bhqk", q, k) * SCALE
    att = np.where(attn_mask[None, None] == 0, -np.inf, att)
    att = att - att.max(axis=-1, keepdims=True)
    att = np.exp(att)
    att = att / att.sum(axis=-1, keepdims=True)
    y = np.einsum("bhqk,bhkd->bhqd", att, v)
    return (y.transpose(0, 2, 1, 3).reshape(B, T, D) @ Wout).astype(np.float32)


def kernel(x, attn_mask, Wqkv, Wout):
    x = np.asarray(x, dtype=np.float32)
    attn_mask = np.asarray(attn_mask)
    Wqkv = np.asarray(Wqkv, dtype=np.float32)
    Wout = np.asarray(Wout, dtype=np.float32)

    causal = bool(
        np.array_equal(attn_mask != 0, np.tril(np.ones((T, T), dtype=bool)))
    )
    if not causal:
        return _reference_fallback(x, attn_mask, Wqkv, Wout)

    out, _ = _run(x, Wqkv, Wout, trace=False)
    return out
